# revision 34
# baseline (speedup 1.0000x reference)
"""CGC (Customized Gate Control) MoE layer on 8 Trainium2 NeuronCores.

Strategy: data-parallel over batch (B=4096 -> 8 shards of BL=512 rows);
every core holds all 8 expert MLPs and computes the full layer for its
shard -- no collectives.

Numerics/speed design (fp8 DoubleRow matmuls run 2 contraction slabs per
pass at 0.5 cyc/row -- 4x the fp32r rate; one e4m3 operand alone is too
lossy for the 2e-2 gate, so compensated schemes are used):
  - L1 (x@W1, 2/3 of FLOPs): fp8e4 "Karatsuba" -- x and W1 are split on
    the host into e4m3 hi+lo pairs (hi=Q(v), lo=Q(v-hi)); three DoubleRow
    matmuls per slab-pair compute hi@hi (2 slabs/pass) + lo@hi + hi@lo.
    Net 0.75 cyc/slab-row (1.33x fp32r) at ~bf16 accuracy.
  - h: single ACT pass per tile: relu(psum/16384 + b1) -> fp16.
  - L2: fp16 x fp16 (h cannot be hi/lo split without doubling the ACT
    drain volume, and a lone e4m3 h fails the error gate). Bias b2 via a
    rank-1 ones x b2 matmul appended to each PSUM group; DVE relu-drains.
  - Gates: gate-weight error multiplies full-size expert outputs, so Wg
    is also hi/lo split; the DoubleRow pair slots carry (x_hi, x_lo)
    against duplicated Wg planes => logits = Wg_hi'(x_hi+x_lo) +
    Wg_lo'(x_hi+x_lo), i.e. fully compensated on both sides.
  - Gated combine: DVE scalar_tensor_tensor MACs on fp16 (2x_1P mode).
  - x is pre-transposed/pre-quantized on the host (input marshalling);
    outputs leave the device as fp16 and are upcast on the host.
Measured end-to-end rel err of this pipeline: ~1.3e-3 (gate: 2e-2).
"""

import numpy as np
import ml_dtypes

import concourse.tile as tile
from concourse import bacc, mybir
from concourse.bass_utils import run_bass_kernel_spmd

N_CORES = 8
B = 4096
BL = B // N_CORES  # 512 rows per core
D = 1024
H1 = 1024
H2 = 512
DOM = 3
NES = 2
NSH = 2
E_SPEC = DOM * NES  # 6
GATE_K = NES + NSH  # 4
TOTAL_E = E_SPEC + NSH  # 8

F32 = mybir.dt.float32
F16 = mybir.dt.float16
FP8 = mybir.dt.float8e4
AX = mybir.AxisListType
AF = mybir.ActivationFunctionType
ALU = mybir.AluOpType
DR = mybir.MatmulPerfMode.DoubleRow

NBT = BL // 128   # 4 batch tiles per core
NKD = D // 128    # 8 contraction slabs over D
NKH = H1 // 128   # 8 contraction slabs over H1
NMH = H1 // 128   # 8 output tiles over H1
NPAIR = NKD // 2  # 4 slab pairs
SCL = 1.0 / 16384.0  # undo host x*16, W*1024 scaling
GPAD = 16  # gate weight columns padded to 16: dual-fp8 Ldweights needs
           # the pair-dim stride % 16 == 0 (s3_lw_dual_fp8_restrictions)

E4NP = ml_dtypes.float8_e4m3


def _build_nc():
    from contextlib import ExitStack

    nc = bacc.Bacc("TRN2", target_bir_lowering=False, debug=False)

    xqs = [
        nc.dram_tensor(n, [D, 2, BL], FP8, kind="ExternalInput")
        for n in ("xq0", "xq1", "xq2", "xqs")
    ]
    W1hi = nc.dram_tensor("W1hi", [TOTAL_E, D, H1], FP8, kind="ExternalInput")
    W1lo = nc.dram_tensor("W1lo", [TOTAL_E, D, H1], FP8, kind="ExternalInput")
    W2f = nc.dram_tensor("W2f", [TOTAL_E, H1, H2], F16, kind="ExternalInput")
    b1f = nc.dram_tensor("b1f", [TOTAL_E, H1], F32, kind="ExternalInput")
    b2q = nc.dram_tensor("b2q", [TOTAL_E, 2, H2], FP8, kind="ExternalInput")
    Wghi = nc.dram_tensor("Wghi", [DOM, D, 2, GPAD], FP8, kind="ExternalInput")
    Wglo = nc.dram_tensor("Wglo", [DOM, D, 2, GPAD], FP8, kind="ExternalInput")
    Wsghi = nc.dram_tensor("Wsghi", [D, 2, GPAD], FP8, kind="ExternalInput")
    Wsglo = nc.dram_tensor("Wsglo", [D, 2, GPAD], FP8, kind="ExternalInput")
    bgf = nc.dram_tensor("bgf", [DOM, GATE_K], F32, kind="ExternalInput")
    bsgf = nc.dram_tensor("bsgf", [TOTAL_E], F32, kind="ExternalInput")
    ys = [
        nc.dram_tensor(n, [BL, H2], F16, kind="ExternalOutput")
        for n in ("y0", "y1", "y2", "ysh")
    ]

    with tile.TileContext(nc) as tc, ExitStack() as ctx:
        p_const = ctx.enter_context(tc.tile_pool(name="const", bufs=1))
        p_xq = ctx.enter_context(tc.tile_pool(name="xq", bufs=2))
        p_w1 = ctx.enter_context(tc.tile_pool(name="w1", bufs=3))
        p_w2 = ctx.enter_context(tc.tile_pool(name="w2", bufs=2))
        p_bias = ctx.enter_context(tc.tile_pool(name="bias", bufs=2))
        p_h = ctx.enter_context(tc.tile_pool(name="hT", bufs=3))
        p_oe = ctx.enter_context(tc.tile_pool(name="oe", bufs=2))
        p_osh = ctx.enter_context(tc.tile_pool(name="osh", bufs=1))
        p_acc = ctx.enter_context(tc.tile_pool(name="acc", bufs=1))
        p_gw = ctx.enter_context(tc.tile_pool(name="gw", bufs=1))
        p_gt = ctx.enter_context(tc.tile_pool(name="gt", bufs=2))
        p_sm = ctx.enter_context(tc.tile_pool(name="sm", bufs=3))
        ps_h = ctx.enter_context(tc.tile_pool(name="psh", bufs=3, space="PSUM"))
        ps_o = ctx.enter_context(tc.tile_pool(name="pso", bufs=3, space="PSUM"))
        ps_t = ctx.enter_context(tc.tile_pool(name="pst", bufs=2, space="PSUM"))

        # Zero tile for PE warm-up first: a single Pool memset, so the first
        # warmup matmul issues ~400ns in instead of waiting on the ident
        # chain.
        zf_sb = p_const.tile([128, 128], F16)
        nc.gpsimd.memset(zf_sb, 0.0)
        # On-chip constants (no DMA ahead of the x/W transfers).
        ident_sb = p_const.tile([128, 128], F32)
        nc.gpsimd.memset(ident_sb, 0.0)
        nc.gpsimd.affine_select(
            out=ident_sb,
            in_=ident_sb,
            compare_op=ALU.not_equal,
            fill=1.0,
            base=0,
            pattern=[[-1, 128]],
            channel_multiplier=1,
        )
        # PE warm-up while the first x/W DMAs are in flight (also walks the
        # pstate ramp so real matmuls run at full clock; sized to bridge to
        # the gate matmuls at ~5us when xq + gate weights have landed).
        for _ in range(56):
            pw = ps_t.tile([128, 128], F32, tag="pt", name="pw")
            nc.tensor.matmul(pw, lhsT=zf_sb, rhs=zf_sb, start=True, stop=True)
        # Rank-1 bias lhsT: both DoubleRow slots hold 2^-9 (the smallest e4m3
        # subnormal); rhs slots carry (hi, lo) of 512*b2, so the pair sums to
        # b2 at natural scale with only second-order quantization error.
        onesf_sb = p_const.tile([1, 256], F32)
        nc.gpsimd.memset(onesf_sb, 1.0 / 512.0)
        ones2_sb = p_const.tile([1, 2, 128], FP8)
        nc.scalar.copy(out=ones2_sb, in_=onesf_sb)

        def load_xq(x_dram):
            xq = p_xq.tile([128, NKD, 2, BL], FP8, tag="xq")
            nc.sync.dma_start(
                out=xq, in_=x_dram[:].rearrange("(kt p) two b -> p kt two b", p=128)
            )
            return xq

        def load_gate_w(wghi_2d, wglo_2d, bias_1d, K):
            wgh = p_sm.tile([128, NKD, 2, GPAD], FP8, tag=f"wgh{K}")
            nc.sync.dma_start(
                out=wgh, in_=wghi_2d.rearrange("(kt p) two k -> p kt two k", p=128)
            )
            wgl = p_sm.tile([128, NKD, 2, GPAD], FP8, tag=f"wgl{K}")
            nc.sync.dma_start(
                out=wgl, in_=wglo_2d.rearrange("(kt p) two k -> p kt two k", p=128)
            )
            bg_sb = p_sm.tile([K, 1], F32, tag=f"bg{K}")
            nc.sync.dma_start(
                out=bg_sb, in_=bias_1d.rearrange("(k one) -> k one", one=1)
            )
            return wgh, wgl, bg_sb

        def gate_logits(xq, wghi_2d, wglo_2d, bias_1d, K, tag, w=None):
            """x @ Wg + bg -> glT tile [K, BL] (logits transposed)."""
            wgh, wgl, bg_sb = w if w is not None else load_gate_w(
                wghi_2d, wglo_2d, bias_1d, K
            )
            pg = ps_t.tile([GPAD, BL], F32, tag="pt")
            for c in range(2):
                cs, ce = c * 256, (c + 1) * 256
                for kt in range(NKD):
                    # slots: (Wg_hi, Wg_hi) x (x_hi, x_lo) then lo plane
                    nc.tensor.matmul(
                        pg[:, cs:ce],
                        lhsT=wgh[:, kt, :, :],
                        rhs=xq[:, kt, :, cs:ce],
                        start=(kt == 0),
                        stop=False,
                        perf_mode=DR,
                    )
                    nc.tensor.matmul(
                        pg[:, cs:ce],
                        lhsT=wgl[:, kt, :, :],
                        rhs=xq[:, kt, :, cs:ce],
                        start=False,
                        stop=(kt == NKD - 1),
                        perf_mode=DR,
                    )
            glT = p_gt.tile([K, BL], F32, tag=f"glT{tag}")
            nc.scalar.activation(
                out=glT, in_=pg[:K, :], func=AF.Identity, bias=bg_sb, scale=SCL
            )
            return glT

        def gate_softmax(glT, K, tag):
            """glT [K, BL] -> softmax over K -> gw [128, NBT, K].

            Emitted via an expert's mid_cb so the PE transposes land after
            ~25us of L1 work and never head-block the PE queue on the ACT
            glT drain."""
            gw = p_gw.tile([128, NBT, K], F32, tag=tag)
            for bt in range(NBT):
                ptg = ps_t.tile([128, K], F32, tag="pt")
                nc.tensor.transpose(
                    ptg, glT[:, bt * 128 : (bt + 1) * 128], ident_sb[:K, :K]
                )
                nm = p_sm.tile([128, 1], F32, tag="nm")
                nc.vector.reduce_max(out=nm, in_=ptg, axis=AX.X, negate=True)
                esb = p_sm.tile([128, K], F32, tag="esb")
                nc.scalar.activation(
                    out=esb, in_=ptg, func=AF.Exp, bias=nm, scale=1.0
                )
                ssb = p_sm.tile([128, 1], F32, tag="ssb")
                nc.vector.reduce_sum(out=ssb, in_=esb, axis=AX.X)
                rsb = p_sm.tile([128, 1], F32, tag="rsb")
                nc.vector.reciprocal(out=rsb, in_=ssb)
                nc.vector.tensor_scalar_mul(gw[:, bt, :], esb, rsb)
            return gw

        def expert(e, xq, out_pool, tag, post_bt=None, mid_cb=None,
                   drain_dve=False):
            """relu(relu(x@W1+b1)@W2+b2) -> [128, NBT, H2] fp16.

            post_bt(bt, oe): emitted right after each batch-tile's relu
            drain so combines/stores pipeline with the remaining tiles.
            mid_cb(): emitted between the L1 and L2 phases (used for the
            gate softmax whose glT input is produced early in L1).
            W1 planes load in column halves so the first L1 matmul only
            waits on half a plane.
            """
            w1h = p_w1.tile([128, NKD, H1], FP8, tag="w1hi")
            w1l = p_w1.tile([128, NKD, H1], FP8, tag="w1lo")
            b1_sb = p_bias.tile([128, NMH], F32, tag="b1")
            b2_sb = p_bias.tile([1, 2, H2], FP8, tag="b2")
            for half in range(2):
                hs, he = half * (H1 // 2), (half + 1) * (H1 // 2)
                nc.sync.dma_start(
                    out=w1h[:, :, hs:he],
                    in_=W1hi[e].rearrange("(kt p) h -> p kt h", p=128)[:, :, hs:he],
                )
                nc.sync.dma_start(
                    out=w1l[:, :, hs:he],
                    in_=W1lo[e].rearrange("(kt p) h -> p kt h", p=128)[:, :, hs:he],
                )
                if half == 0:
                    # biases ride between the half-pairs: in time for the
                    # first ACT drain, not ahead of the hot W1 pieces
                    nc.sync.dma_start(
                        out=b1_sb, in_=b1f[e].rearrange("(mt p) -> p mt", p=128)
                    )
                    nc.sync.dma_start(out=b2_sb, in_=b2q[e])
            w2_sb = p_w2.tile([128, NKH, H2], F16, tag="w2")
            nc.sync.dma_start(
                out=w2_sb, in_=W2f[e].rearrange("(kt p) o -> p kt o", p=128)
            )
            hT = p_h.tile([128, NMH, BL], F16, tag="hT")
            for mt in range(NMH):
                ms, me = mt * 128, (mt + 1) * 128
                ph = ps_h.tile([128, BL], F32, tag="ph")
                for c in range(2):
                    cs, ce = c * 256, (c + 1) * 256
                    # hi-terms first: the first matmuls of the kernel then
                    # depend only on the W1hi DMA, not W1lo.
                    for j in range(NPAIR):  # hi(W) @ hi(x), 2 slabs per pass
                        nc.tensor.matmul(
                            ph[:, cs:ce],
                            lhsT=w1h[:, 2 * j : 2 * j + 2, ms:me],
                            rhs=xq[:, 2 * j : 2 * j + 2, 0, cs:ce],
                            start=(j == 0),
                            stop=False,
                            perf_mode=DR,
                        )
                    for j in range(NPAIR):  # lo(W) @ hi(x)
                        nc.tensor.matmul(
                            ph[:, cs:ce],
                            lhsT=w1l[:, 2 * j : 2 * j + 2, ms:me],
                            rhs=xq[:, 2 * j : 2 * j + 2, 0, cs:ce],
                            start=False,
                            stop=False,
                            perf_mode=DR,
                        )
                    for j in range(NPAIR):  # hi(W) @ lo(x)
                        nc.tensor.matmul(
                            ph[:, cs:ce],
                            lhsT=w1h[:, 2 * j : 2 * j + 2, ms:me],
                            rhs=xq[:, 2 * j : 2 * j + 2, 1, cs:ce],
                            start=False,
                            stop=(j == NPAIR - 1),
                            perf_mode=DR,
                        )
                nc.scalar.activation(
                    out=hT[:, mt, :],
                    in_=ph,
                    func=AF.Relu,
                    bias=b1_sb[:, mt : mt + 1],
                    scale=SCL,
                )
            if mid_cb is not None:
                mid_cb()
            oe = out_pool.tile([128, NBT, H2], F16, tag=tag)
            for bt in range(NBT):
                bs, be = bt * 128, (bt + 1) * 128
                po = ps_o.tile([128, H2], F32, tag="po")
                for kt in range(NKH):
                    nc.tensor.matmul(
                        po,
                        lhsT=hT[:, kt, bs:be],
                        rhs=w2_sb[:, kt, :],
                        start=(kt == 0),
                        stop=False,
                    )
                for c in range(2):
                    cs, ce = c * 256, (c + 1) * 256
                    nc.tensor.matmul(
                        po[:, cs:ce],
                        lhsT=ones2_sb,
                        rhs=b2_sb[:, :, cs:ce],
                        start=False,
                        stop=True,
                        perf_mode=DR,
                    )
                # Relu-drain on ACT (DVE is the combine engine; keeping the
                # drain off it lets the per-bt combines keep pace with PE).
                # The final expert drains on DVE: its combines are the
                # kernel tail and this skips the ACT->DVE handoff there.
                if drain_dve:
                    nc.vector.tensor_scalar_max(oe[:, bt, :], po, 0.0)
                else:
                    nc.scalar.activation(out=oe[:, bt, :], in_=po, func=AF.Relu)
                if post_bt is not None:
                    post_bt(bt, oe)
            return oe

        accs = [None] * 4

        def acc_bt(acc_idx, oe, bt, gw, col, first=False):
            acc = accs[acc_idx]
            if first:
                nc.vector.tensor_scalar_mul(
                    acc[:, bt, :], oe[:, bt, :], gw[:, bt, col : col + 1]
                )
            else:
                nc.vector.scalar_tensor_tensor(
                    out=acc[:, bt, :],
                    in0=oe[:, bt, :],
                    scalar=gw[:, bt, col : col + 1],
                    in1=acc[:, bt, :],
                    op0=ALU.mult,
                    op1=ALU.add,
                )

        def accumulate(acc_idx, oe, gw, col, first):
            for bt in range(NBT):
                acc_bt(acc_idx, oe, bt, gw, col, first)

        yrs = [y[:].rearrange("(bt p) o -> bt p o", p=128) for y in ys]

        # ---- shared phase (gate logits first: they only need xq + 40KB of
        # gate weights, so they give PE work while the W1 planes stream in;
        # the softmax transposes ride expert 6's mid_cb so they never wait
        # on the ACT glT drain at the head of the PE queue) ----
        w_sh = load_gate_w(Wsghi[:], Wsglo[:], bsgf[:], TOTAL_E)  # tiny, first
        xq_cur = load_xq(xqs[3])
        glT_s = gate_logits(
            xq_cur, Wsghi[:], Wsglo[:], bsgf[:], TOTAL_E, "gws", w=w_sh
        )
        hold = {}

        def mid_sh():
            hold["gws"] = gate_softmax(glT_s, TOTAL_E, tag="gws")

        osh = [expert(E_SPEC, xq_cur, p_osh, tag="osh0", mid_cb=mid_sh)]
        osh.append(expert(E_SPEC + 1, xq_cur, p_osh, tag="osh1"))
        gws = hold["gws"]
        xq_next = load_xq(xqs[0])
        accs[3] = p_acc.tile([128, NBT, H2], F16, tag="acc3", name="acc3")
        accumulate(3, osh[0], gws, E_SPEC + 0, first=True)
        accumulate(3, osh[1], gws, E_SPEC + 1, first=False)

        # ---- domain phases ----
        for d in range(DOM):
            xq_cur = xq_next
            glT_d = gate_logits(
                xq_cur, Wghi[d], Wglo[d], bgf[d], GATE_K, tag=f"gw{d}"
            )
            accs[d] = p_acc.tile(
                [128, NBT, H2], F16, tag=f"acc{d}", name=f"acc{d}"
            )

            def mid_d(d=d, glT_d=glT_d):
                gw = hold[f"gw{d}"] = gate_softmax(glT_d, GATE_K, tag=f"gw{d}")
                accumulate(d, osh[0], gw, NES + 0, first=True)
                accumulate(d, osh[1], gw, NES + 1, first=False)

            for i in range(NES):
                e = d * NES + i
                last_dom = i == NES - 1
                last_all = e == E_SPEC - 1

                def post(bt, oe, d=d, i=i, e=e,
                         last_dom=last_dom, last_all=last_all):
                    # acc3 first: on the final expert the ysh store chain
                    # (the kernel's critical tail) starts one stt earlier.
                    acc_bt(3, oe, bt, gws, e)
                    if last_all:
                        nc.sync.dma_start(out=yrs[3][bt], in_=accs[3][:, bt, :])
                    acc_bt(d, oe, bt, hold[f"gw{d}"], i)
                    if last_dom:
                        nc.sync.dma_start(out=yrs[d][bt], in_=accs[d][:, bt, :])

                oe = expert(e, xq_cur, p_oe, tag="oe",
                            post_bt=post, mid_cb=mid_d if i == 0 else None,
                            drain_dve=last_all)
                if i == 0 and d < DOM - 1:
                    xq_next = load_xq(xqs[d + 1])

    nc.compile()
    return nc


_NC_CACHE = {}


def _get_nc():
    if "nc" not in _NC_CACHE:
        _NC_CACHE["nc"] = _build_nc()
    return _NC_CACHE["nc"]


def _split8(v, scale):
    """v -> (hi, lo) e4m3 pair with hi = Q(v*scale), lo = Q(v*scale - hi)."""
    v = np.asarray(v, np.float32) * scale
    hi = v.astype(E4NP)
    lo = (v - hi.astype(np.float32)).astype(E4NP)
    return hi, lo


def kernel(**inputs):
    return run_kernel(inputs)


def run_kernel(inputs, trace=False):
    nc = _get_nc()
    f = {k: np.asarray(v, dtype=np.float32) for k, v in inputs.items()}

    W1all = np.concatenate([f["W1s"], f["W1h"]])            # [8, D, H1]
    W2all = np.concatenate([f["W2s"], f["W2h"]])            # [8, H1, H2]
    b1all = np.ascontiguousarray(np.concatenate([f["b1s"], f["b1h"]]))
    b2hi, b2lo = _split8(np.concatenate([f["b2s"], f["b2h"]]), 512.0)
    b2q = np.ascontiguousarray(np.stack([b2hi, b2lo], axis=1))  # [8, 2, H2]
    w1hi, w1lo = _split8(W1all, 1024.0)
    w1hi, w1lo = np.ascontiguousarray(w1hi), np.ascontiguousarray(w1lo)
    w2f = np.ascontiguousarray(W2all.astype(np.float16))

    def dup_gate(w):  # [D, K] -> [D, 2, GPAD] dup'd planes, zero-padded cols
        out = np.zeros((w.shape[0], 2, GPAD), dtype=w.dtype)
        out[:, 0, : w.shape[1]] = w
        out[:, 1, : w.shape[1]] = w
        return out

    wghi, wglo = _split8(f["Wg"], 1024.0)                   # [DOM, D, K]
    wghi = np.stack([dup_gate(wghi[d]) for d in range(DOM)])
    wglo = np.stack([dup_gate(wglo[d]) for d in range(DOM)])
    wsghi, wsglo = _split8(f["Wsg"], 1024.0)                # [D, 8]
    wsghi, wsglo = dup_gate(wsghi), dup_gate(wsglo)

    shared = {
        "W1hi": w1hi, "W1lo": w1lo, "W2f": w2f,
        "b1f": b1all, "b2q": b2q,
        "Wghi": wghi, "Wglo": wglo, "Wsghi": wsghi, "Wsglo": wsglo,
        "bgf": np.ascontiguousarray(f["bg"]),
        "bsgf": np.ascontiguousarray(f["bsg"]),
    }

    xnames = ("x0", "x1", "x2", "x_shared")
    qnames = ("xq0", "xq1", "xq2", "xqs")
    in_maps = []
    for c in range(N_CORES):
        m = dict(shared)
        for xn, qn in zip(xnames, qnames):
            shard = f[xn][c * BL : (c + 1) * BL]            # [BL, D]
            hi, lo = _split8(shard.T, 16.0)                 # [D, BL] each
            m[qn] = np.ascontiguousarray(np.stack([hi, lo], axis=1))
        in_maps.append(m)

    res = run_bass_kernel_spmd(nc, in_maps, list(range(N_CORES)), trace=trace)
    outs = []
    for name in ("y0", "y1", "y2", "ysh"):
        outs.append(
            np.concatenate(
                [np.asarray(res.results[c][name]) for c in range(N_CORES)], axis=0
            ).astype(np.float32)
        )
    out = tuple(outs)
    if trace:
        return out, res
    return out
